# revision 55
# baseline (speedup 1.0000x reference)
"""JoinConvNet Trainium2 kernel — 8-core data-parallel, raw Bass.

Math per subnet (reference):
  conv(x, w)  : x[B,1,L,E], w[C,1,W,E], VALID -> c[B,C,L-W+1]
  m = max_l c ; h = relu(m + b_conv) ; o = relu(h @ w_fc.T + b_fc)
  out[b] = dot(o1[b], o2[b])

Device strategy (per core, 128 batches):
  The e[0:256) part of the contraction runs in fp8e4m3 with
  MatmulPerfMode.DoubleRow: host packs pairs (x[e], x[e+128]) interleaved
  along the moving free dim and weights as [K,2,M] pair tiles, so one
  DoubleRow matmul per tap contracts 256 e-values at 0.5 cycles/row.
  The e[256:300) leftover runs in fp16 via the shared 88-row plane
  (x1 rows 0:44, x2 rows 44:88, zero-padded stationary) with the tap
  shift applied through the moving-column offset.

  Per (group of 400 positions, subnet): 3 DoubleRow + 3 fp16 accumulating
  matmuls -> PSUM [80,400] -> per-batch reduce_max -> H[80,128] ->
  bias+relu -> FC matmul -> bias+relu -> elementwise mul -> ones-matmul
  partition sum -> [1,128] out.

  Startup is bootstrapped with small leading DMAs so the PE starts early.
  fp8 end-to-end rel err vs the fp32 reference: ~1.5e-2 (measured, fixed
  inputs), under the 2e-2 gate.
"""
import os
import numpy as np
import ml_dtypes
from contextlib import ExitStack

import concourse.bass as bass
import concourse.mybir as mybir
from concourse.bass_utils import run_bass_kernel_spmd

B, L, E = 1024, 200, 300
C, W, O = 80, 3, 30
NCORES = 8
BS = B // NCORES            # 128 batches/core
POS = BS * L                # 25600 positions/core
PADPOS = POS + 2
NSLAB = int(os.environ.get("K_NSLAB", "32"))
SLAB = POS // NSLAB         # positions per slab
SLABW = SLAB + 2            # loaded columns per slab
NG = SLAB // 400            # 400-position groups per slab
SLOTS = int(os.environ.get("K_SLOTS", "4"))
GN = 400                    # matmul moving size
VALID = L - W + 1           # 198
BOOT = bool(int(os.environ.get("K_BOOT", "1")))

F8 = mybir.dt.float8e4
F16 = mybir.dt.float16
F32 = mybir.dt.float32
NP_F8 = ml_dtypes.float8_e4m3

LAST_RESULT = None
TRACE = bool(os.environ.get("KERNEL_TRACE"))
_NC_CACHE = {}


SLABW8 = (SLABW + 15) // 16 * 16  # pair-plane stride must be 16B-aligned
WSCALE = 64.0  # lift conv weights (~0.02) out of the e4m3 subnormal range


def _build_nc():
    nc = bass.Bass()
    xq8 = nc.declare_dram_parameter("xq8", [128, 2, 2, PADPOS], F8, isOutput=False)
    xp2 = nc.declare_dram_parameter("xp2", [88, PADPOS], F16, isOutput=False)
    wst8 = nc.declare_dram_parameter("wst8", [128, 6, 2, C], F8, isOutput=False)
    wst16 = nc.declare_dram_parameter("wst16", [88, 6 * C], F16, isOutput=False)
    wf = nc.declare_dram_parameter("wf", [C, 2 * O], F32, isOutput=False)
    bc = nc.declare_dram_parameter("bc", [C, 2], F32, isOutput=False)
    bf = nc.declare_dram_parameter("bf", [O, 2], F32, isOutput=False)
    out = nc.declare_dram_parameter("out", [1, BS], F32, isOutput=True)

    with ExitStack() as ctx:
        X8 = ctx.enter_context(nc.sbuf_tensor([128, SLOTS, 2, 2, SLABW8], F8))
        X2 = ctx.enter_context(nc.sbuf_tensor([88, SLOTS, SLABW], F16))
        Wc8 = ctx.enter_context(nc.sbuf_tensor([128, 6, 2, C], F8))
        Wc16 = ctx.enter_context(nc.sbuf_tensor([88, 6 * C], F16))
        Wf = ctx.enter_context(nc.sbuf_tensor([C, 2 * O], F32))
        Bc = ctx.enter_context(nc.sbuf_tensor([C, 2], F32))
        Bf = ctx.enter_context(nc.sbuf_tensor([O, 2], F32))
        ones = ctx.enter_context(nc.sbuf_tensor([O, 1], F32))
        H = ctx.enter_context(nc.sbuf_tensor([C, 2, BS], F32))
        Hr = ctx.enter_context(nc.sbuf_tensor([C, 2, BS], F32))
        Ofc = ctx.enter_context(nc.sbuf_tensor([O, 2, BS], F32))
        P = ctx.enter_context(nc.sbuf_tensor([O, BS], F32))
        osb = ctx.enter_context(nc.sbuf_tensor([1, BS], F32))
        cps = [ctx.enter_context(nc.psum_tensor(f"cps{i}", [C, GN], F32)) for i in range(4)]
        fps = [ctx.enter_context(nc.psum_tensor(f"fps{i}", [O, BS], F32)) for i in range(2)]
        dps = ctx.enter_context(nc.psum_tensor([1, BS], F32))

        dma_sem = ctx.enter_context(nc.semaphore("dma_sem"))
        dma2_sem = ctx.enter_context(nc.semaphore("dma2_sem"))
        pe_sem = ctx.enter_context(nc.semaphore("pe_sem"))
        red_sem = ctx.enter_context(nc.semaphore("red_sem"))
        act_sem = ctx.enter_context(nc.semaphore("act_sem"))
        fc_sem = ctx.enter_context(nc.semaphore("fc_sem"))
        block = ctx.enter_context(nc.Block())

        # sync queue: 4 boot DMAs (64), X8 slab s at 64+16(s+1); wf/bc/bf
        # (+48) interleaved after slab 2.
        NB1 = 64 if BOOT else 0
        NB2 = 32 if BOOT else 16

        def slab_ready(s):
            return NB1 + 16 * (s + 1) + (48 if s >= 3 else 0)

        # scalar queue: boot DMAs, X2 slab s at NB2+16(s+1)
        def slab2_ready(s):
            return NB2 + 16 * (s + 1)

        @block.sync
        def _(sync):
            # bootstrap: first-group slices so the PE starts early. Boot and
            # slab-0 main DMAs cover DISJOINT column ranges: a DMA re-writing
            # bytes the PE is reading corrupts them even with identical data.
            if BOOT:
                sync.dma_start(out=X8[:, 0, 0, :, 0:402], in_=xq8[:, 0, :, 0:402]).then_inc(dma_sem, 16)
                sync.dma_start(out=Wc8[:, 0:3, :, :], in_=wst8[:, 0:3, :, :]).then_inc(dma_sem, 16)
                sync.dma_start(out=X8[:, 0, 1, :, 0:402], in_=xq8[:, 1, :, 0:402]).then_inc(dma_sem, 16)
                sync.dma_start(out=Wc8[:, 3:6, :, :], in_=wst8[:, 3:6, :, :]).then_inc(dma_sem, 16)
            else:
                sync.dma_start(out=Wc8[:, :, :, :], in_=wst8[:, :, :, :]).then_inc(dma_sem, 16)
            for s in range(NSLAB):
                if s == 3:
                    sync.dma_start(out=Wf[:, :], in_=wf[:, :]).then_inc(dma_sem, 16)
                    sync.dma_start(out=Bc[:, :], in_=bc[:, :]).then_inc(dma_sem, 16)
                    sync.dma_start(out=Bf[:, :], in_=bf[:, :]).then_inc(dma_sem, 16)
                if s >= SLOTS:
                    # slot s%SLOTS free once PE finished slab s-SLOTS
                    sync.wait_ge(pe_sem, 2 * NG * (s - SLOTS + 1))
                lo = 402 if (s == 0 and BOOT) else 0
                sync.dma_start(
                    out=X8[:, s % SLOTS, :, :, lo:SLABW],
                    in_=xq8[:, :, :, s * SLAB + lo : s * SLAB + SLABW],
                ).then_inc(dma_sem, 16)
            sync.wait_ge(act_sem, 3)
            sync.dma_start(out=out[:, :], in_=osb[:, :]).then_inc(dma_sem, 16)

        @block.tensor
        def _(tensor):
            k = 0
            for s in range(NSLAB):
                if s >= 1:
                    tensor.wait_ge(dma_sem, slab_ready(s))
                    tensor.wait_ge(dma2_sem, slab2_ready(s))
                for g in range(NG):
                    for n in range(2):
                        if s == 0 and g == 0 and n == 0:
                            if BOOT:
                                tensor.wait_ge(dma_sem, 64)
                                tensor.wait_ge(dma2_sem, 32)
                            else:
                                tensor.wait_ge(dma_sem, slab_ready(0))
                                tensor.wait_ge(dma2_sem, slab2_ready(0))
                        if BOOT and s == 0 and g == 1 and n == 0:
                            tensor.wait_ge(dma_sem, slab_ready(0))
                            tensor.wait_ge(dma2_sem, slab2_ready(0))
                        if k >= 4:
                            tensor.wait_ge(red_sem, k - 3)
                        # 3 fp8 DoubleRow matmuls (e[0:256), tap via offset)
                        for w in range(W):
                            cl = g * GN + w
                            tensor.matmul(
                                cps[k % 4][:, :],
                                Wc8[:, 3 * n + w, :, :],
                                X8[:, s % SLOTS, n, :, cl : cl + GN],
                                start=(w == 0),
                                stop=False,
                                perf_mode=mybir.MatmulPerfMode.DoubleRow,
                            )
                        # 3 fp16 leftover matmuls (e[256:300), shared plane)
                        for w in range(W):
                            col = (3 * n + w) * C
                            mm = tensor.matmul(
                                cps[k % 4][:, :],
                                Wc16[:, col : col + C],
                                X2[:, s % SLOTS, g * GN + w : g * GN + w + GN],
                                start=False,
                                stop=(w == 2),
                            )
                        mm.then_inc(pe_sem, 1)
                        k += 1
            # FC + dot tail
            tensor.wait_ge(act_sem, 1)
            tensor.matmul(fps[0][:, :], Wf[:, 0:O], Hr[:, 0, :], start=True, stop=True)
            tensor.matmul(
                fps[1][:, :], Wf[:, O : 2 * O], Hr[:, 1, :], start=True, stop=True
            ).then_inc(fc_sem, 1)
            tensor.wait_ge(red_sem, 2 * NG * NSLAB + 1)
            tensor.matmul(dps[:, :], ones[:, :], P[:, :], start=True, stop=True).then_inc(
                fc_sem, 1
            )

        @block.vector
        def _(vector):
            vector.memset(ones[:, :], 1.0)
            k = 0
            for s in range(NSLAB):
                for g in range(NG):
                    p = s * NG + g  # batch pair index
                    for n in range(2):
                        vector.wait_ge(pe_sem, k + 1)
                        vector.reduce_max(
                            H[:, n, 2 * p : 2 * p + 1],
                            cps[k % 4][:, 0:VALID],
                            axis=mybir.AxisListType.X,
                        )
                        vector.reduce_max(
                            H[:, n, 2 * p + 1 : 2 * p + 2],
                            cps[k % 4][:, 200 : 200 + VALID],
                            axis=mybir.AxisListType.X,
                        ).then_inc(red_sem, 1)
                        k += 1
            vector.wait_ge(act_sem, 2)
            vector.tensor_mul(P[:, :], Ofc[:, 0, :], Ofc[:, 1, :]).then_inc(red_sem, 1)

        @block.scalar
        def _(scalar):
            # second DGE queue: fp16 leftover planes (+ disjoint boot slices)
            if BOOT:
                scalar.dma_start(out=X2[:, 0, 0:402], in_=xp2[:, 0:402]).then_inc(dma2_sem, 16)
            scalar.dma_start(out=Wc16[:, :], in_=wst16[:, :]).then_inc(dma2_sem, 16)
            for s in range(NSLAB):
                if s >= SLOTS:
                    scalar.wait_ge(pe_sem, 2 * NG * (s - SLOTS + 1))
                lo = 402 if (s == 0 and BOOT) else 0
                scalar.dma_start(
                    out=X2[:, s % SLOTS, lo:SLABW],
                    in_=xp2[:, s * SLAB + lo : s * SLAB + SLABW],
                ).then_inc(dma2_sem, 16)
            scalar.wait_ge(red_sem, 2 * NG * NSLAB)
            scalar.wait_ge(dma_sem, 160)
            scalar.activation(
                Hr[:, 0, :], H[:, 0, :], mybir.ActivationFunctionType.Relu,
                bias=Bc[:, 0:1], scale=1.0 / WSCALE,
            )
            scalar.activation(
                Hr[:, 1, :], H[:, 1, :], mybir.ActivationFunctionType.Relu,
                bias=Bc[:, 1:2], scale=1.0 / WSCALE,
            ).then_inc(act_sem, 1)
            scalar.wait_ge(fc_sem, 1)
            scalar.activation(
                Ofc[:, 0, :], fps[0][:, :], mybir.ActivationFunctionType.Relu,
                bias=Bf[:, 0:1],
            )
            scalar.activation(
                Ofc[:, 1, :], fps[1][:, :], mybir.ActivationFunctionType.Relu,
                bias=Bf[:, 1:2],
            ).then_inc(act_sem, 1)
            scalar.wait_ge(fc_sem, 2)
            scalar.copy(osb[:, :], dps[:, :]).then_inc(act_sem, 1)

    return nc


def _prep_weights(w_conv1, w_conv2, w_fc1, w_fc2, b_conv1, b_conv2, b_fc1, b_fc2):
    wst8 = np.zeros((128, 6, 2, C), dtype=np.float32)
    wst16 = np.zeros((88, 6, C), dtype=np.float32)
    for n, wc in enumerate((w_conv1, w_conv2)):
        wcs = wc[:, 0] * WSCALE  # [C, W, E]
        for w in range(W):
            blk = 3 * n + w
            wst8[:, blk, 0, :] = wcs[:, w, 0:128].T
            wst8[:, blk, 1, :] = wcs[:, w, 128:256].T
            if n == 0:
                wst16[0:44, blk, :] = wcs[:, w, 256:300].T
            else:
                wst16[44:88, blk, :] = wcs[:, w, 256:300].T
    wf = np.concatenate([w_fc1.T, w_fc2.T], axis=1).astype(np.float32)  # [C, 2O]
    bc = np.stack([b_conv1, b_conv2], axis=1).astype(np.float32)  # [C, 2]
    bf = np.stack([b_fc1, b_fc2], axis=1).astype(np.float32)  # [O, 2]
    return (
        wst8.astype(NP_F8),
        wst16.reshape(88, 6 * C).astype(np.float16),
        wf, bc, bf,
    )


def kernel(x1, x2, w_conv1, b_conv1, w_fc1, b_fc1, w_conv2, b_conv2, w_fc2, b_fc2):
    global LAST_RESULT
    x1 = np.asarray(x1, dtype=np.float32)
    x2 = np.asarray(x2, dtype=np.float32)
    wst8, wst16, wf, bc, bf = _prep_weights(
        np.asarray(w_conv1, np.float32), np.asarray(w_conv2, np.float32),
        np.asarray(w_fc1, np.float32), np.asarray(w_fc2, np.float32),
        np.asarray(b_conv1, np.float32), np.asarray(b_conv2, np.float32),
        np.asarray(b_fc1, np.float32), np.asarray(b_fc2, np.float32),
    )

    if "nc" not in _NC_CACHE:
        _NC_CACHE["nc"] = _build_nc()
    nc = _NC_CACHE["nc"]

    in_maps = []
    for c in range(NCORES):
        xs1 = x1[c * BS : (c + 1) * BS, 0].reshape(POS, E).T  # [300, POS] f32
        xs2 = x2[c * BS : (c + 1) * BS, 0].reshape(POS, E).T
        xq8 = np.zeros((128, 2, 2, PADPOS), dtype=NP_F8)
        for n, xs in enumerate((xs1, xs2)):
            xq8[:, n, 0, :POS] = xs[0:128].astype(NP_F8)
            xq8[:, n, 1, :POS] = xs[128:256].astype(NP_F8)
        xp2 = np.zeros((88, PADPOS), dtype=np.float16)
        xp2[0:44, :POS] = xs1[256:300]
        xp2[44:88, :POS] = xs2[256:300]
        in_maps.append(
            {"xq8": xq8, "xp2": xp2, "wst8": wst8, "wst16": wst16,
             "wf": wf, "bc": bc, "bf": bf}
        )

    res = run_bass_kernel_spmd(nc, in_maps, list(range(NCORES)), trace=TRACE)
    LAST_RESULT = res
    return np.concatenate(
        [res.results[c]["out"].reshape(BS, 1) for c in range(NCORES)], axis=0
    )


# revision 56
# speedup vs baseline: 1.0034x; 1.0034x over previous
"""JoinConvNet Trainium2 kernel — 8-core data-parallel, raw Bass.

Math per subnet (reference):
  conv(x, w)  : x[B,1,L,E], w[C,1,W,E], VALID -> c[B,C,L-W+1]
  m = max_l c ; h = relu(m + b_conv) ; o = relu(h @ w_fc.T + b_fc)
  out[b] = dot(o1[b], o2[b])

Device strategy (per core, 128 batches):
  The e[0:256) part of the contraction runs in fp8e4m3 with
  MatmulPerfMode.DoubleRow: host packs pairs (x[e], x[e+128]) interleaved
  along the moving free dim and weights as [K,2,M] pair tiles, so one
  DoubleRow matmul per tap contracts 256 e-values at 0.5 cycles/row.
  The e[256:300) leftover runs in fp16 via the shared 88-row plane
  (x1 rows 0:44, x2 rows 44:88, zero-padded stationary) with the tap
  shift applied through the moving-column offset.

  Per (group of 400 positions, subnet): 3 DoubleRow + 3 fp16 accumulating
  matmuls -> PSUM [80,400] -> per-batch reduce_max -> H[80,128] ->
  bias+relu -> FC matmul -> bias+relu -> elementwise mul -> ones-matmul
  partition sum -> [1,128] out.

  Startup is bootstrapped with small leading DMAs so the PE starts early.
  fp8 end-to-end rel err vs the fp32 reference: ~1.5e-2 (measured, fixed
  inputs), under the 2e-2 gate.
"""
import os
import numpy as np
import ml_dtypes
from contextlib import ExitStack

import concourse.bass as bass
import concourse.mybir as mybir
from concourse.bass_utils import run_bass_kernel_spmd

B, L, E = 1024, 200, 300
C, W, O = 80, 3, 30
NCORES = 8
BS = B // NCORES            # 128 batches/core
POS = BS * L                # 25600 positions/core
PADPOS = POS + 2
NSLAB = int(os.environ.get("K_NSLAB", "32"))
SLAB = POS // NSLAB         # positions per slab
SLABW = SLAB + 2            # loaded columns per slab
NG = SLAB // 400            # 400-position groups per slab
SLOTS = int(os.environ.get("K_SLOTS", "4"))
GN = 400                    # matmul moving size
VALID = L - W + 1           # 198
BOOT = bool(int(os.environ.get("K_BOOT", "1")))

F8 = mybir.dt.float8e4
F16 = mybir.dt.float16
F32 = mybir.dt.float32
NP_F8 = ml_dtypes.float8_e4m3

LAST_RESULT = None
TRACE = bool(os.environ.get("KERNEL_TRACE"))
_NC_CACHE = {}


SLABW8 = (SLABW + 15) // 16 * 16  # pair-plane stride must be 16B-aligned
WSCALE = 64.0  # lift conv weights (~0.02) out of the e4m3 subnormal range


def _build_nc():
    nc = bass.Bass()
    xq8 = nc.declare_dram_parameter("xq8", [128, 2, 2, PADPOS], F8, isOutput=False)
    xp2 = nc.declare_dram_parameter("xp2", [88, PADPOS], F16, isOutput=False)
    wst8 = nc.declare_dram_parameter("wst8", [128, 6, 2, C], F8, isOutput=False)
    wst16 = nc.declare_dram_parameter("wst16", [88, 6 * C], F16, isOutput=False)
    wf = nc.declare_dram_parameter("wf", [C, 2 * O], F32, isOutput=False)
    bc = nc.declare_dram_parameter("bc", [C, 2], F32, isOutput=False)
    bf = nc.declare_dram_parameter("bf", [O, 2], F32, isOutput=False)
    out = nc.declare_dram_parameter("out", [1, BS], F32, isOutput=True)

    with ExitStack() as ctx:
        X8 = ctx.enter_context(nc.sbuf_tensor([128, SLOTS, 2, 2, SLABW8], F8))
        X2 = ctx.enter_context(nc.sbuf_tensor([88, SLOTS, SLABW], F16))
        Wc8 = ctx.enter_context(nc.sbuf_tensor([128, 6, 2, C], F8))
        Wc16 = ctx.enter_context(nc.sbuf_tensor([88, 6 * C], F16))
        Wf = ctx.enter_context(nc.sbuf_tensor([C, 2 * O], F32))
        Bc = ctx.enter_context(nc.sbuf_tensor([C, 2], F32))
        Bf = ctx.enter_context(nc.sbuf_tensor([O, 2], F32))
        ones = ctx.enter_context(nc.sbuf_tensor([O, 1], F32))
        H = ctx.enter_context(nc.sbuf_tensor([C, 2, BS], F32))
        Hr = ctx.enter_context(nc.sbuf_tensor([C, 2, BS], F32))
        Ofc = ctx.enter_context(nc.sbuf_tensor([O, 2, BS], F32))
        P = ctx.enter_context(nc.sbuf_tensor([O, BS], F32))
        osb = ctx.enter_context(nc.sbuf_tensor([1, BS], F32))
        cps = [ctx.enter_context(nc.psum_tensor(f"cps{i}", [C, GN], F32)) for i in range(4)]
        fps = [ctx.enter_context(nc.psum_tensor(f"fps{i}", [O, BS], F32)) for i in range(2)]
        dps = ctx.enter_context(nc.psum_tensor([1, BS], F32))

        dma_sem = ctx.enter_context(nc.semaphore("dma_sem"))
        dma2_sem = ctx.enter_context(nc.semaphore("dma2_sem"))
        pe_sem = ctx.enter_context(nc.semaphore("pe_sem"))
        red_sem = ctx.enter_context(nc.semaphore("red_sem"))
        act_sem = ctx.enter_context(nc.semaphore("act_sem"))
        fc_sem = ctx.enter_context(nc.semaphore("fc_sem"))
        block = ctx.enter_context(nc.Block())

        # sync queue: 4 boot DMAs (64), X8 slab s at 64+16(s+1); wf/bc/bf
        # (+48) interleaved after slab 2.
        NB1 = 64 if BOOT else 0
        NB2 = 32 if BOOT else 16

        def slab_ready(s):
            return NB1 + 16 * (s + 1) + (48 if s >= 3 else 0)

        # scalar queue: boot DMAs, X2 slab s at NB2+16(s+1)
        def slab2_ready(s):
            return NB2 + 16 * (s + 1)

        @block.sync
        def _(sync):
            # bootstrap: first-group slices so the PE starts early. Boot and
            # slab-0 main DMAs cover DISJOINT column ranges: a DMA re-writing
            # bytes the PE is reading corrupts them even with identical data.
            if BOOT:
                sync.dma_start(out=X8[:, 0, 0, :, 0:402], in_=xq8[:, 0, :, 0:402]).then_inc(dma_sem, 16)
                sync.dma_start(out=Wc8[:, 0:3, :, :], in_=wst8[:, 0:3, :, :]).then_inc(dma_sem, 16)
                sync.dma_start(out=X8[:, 0, 1, :, 0:402], in_=xq8[:, 1, :, 0:402]).then_inc(dma_sem, 16)
                sync.dma_start(out=Wc8[:, 3:6, :, :], in_=wst8[:, 3:6, :, :]).then_inc(dma_sem, 16)
            else:
                sync.dma_start(out=Wc8[:, :, :, :], in_=wst8[:, :, :, :]).then_inc(dma_sem, 16)
            for s in range(NSLAB):
                if s == 3:
                    sync.dma_start(out=Wf[:, :], in_=wf[:, :]).then_inc(dma_sem, 16)
                    sync.dma_start(out=Bc[:, :], in_=bc[:, :]).then_inc(dma_sem, 16)
                    sync.dma_start(out=Bf[:, :], in_=bf[:, :]).then_inc(dma_sem, 16)
                if s >= SLOTS:
                    # slot s%SLOTS free once PE finished slab s-SLOTS
                    sync.wait_ge(pe_sem, 2 * NG * (s - SLOTS + 1))
                lo = 402 if (s == 0 and BOOT) else 0
                sync.dma_start(
                    out=X8[:, s % SLOTS, :, :, lo:SLABW],
                    in_=xq8[:, :, :, s * SLAB + lo : s * SLAB + SLABW],
                ).then_inc(dma_sem, 16)


        @block.tensor
        def _(tensor):
            k = 0
            for s in range(NSLAB):
                if s >= 1:
                    tensor.wait_ge(dma_sem, slab_ready(s))
                    tensor.wait_ge(dma2_sem, slab2_ready(s))
                for g in range(NG):
                    for n in range(2):
                        if s == 0 and g == 0 and n == 0:
                            if BOOT:
                                tensor.wait_ge(dma_sem, 64)
                                tensor.wait_ge(dma2_sem, 32)
                            else:
                                tensor.wait_ge(dma_sem, slab_ready(0))
                                tensor.wait_ge(dma2_sem, slab2_ready(0))
                        if BOOT and s == 0 and g == 1 and n == 0:
                            tensor.wait_ge(dma_sem, slab_ready(0))
                            tensor.wait_ge(dma2_sem, slab2_ready(0))
                        if k >= 4:
                            tensor.wait_ge(red_sem, k - 3)
                        # 3 fp8 DoubleRow matmuls (e[0:256), tap via offset)
                        for w in range(W):
                            cl = g * GN + w
                            tensor.matmul(
                                cps[k % 4][:, :],
                                Wc8[:, 3 * n + w, :, :],
                                X8[:, s % SLOTS, n, :, cl : cl + GN],
                                start=(w == 0),
                                stop=False,
                                perf_mode=mybir.MatmulPerfMode.DoubleRow,
                            )
                        # 3 fp16 leftover matmuls (e[256:300), shared plane)
                        for w in range(W):
                            col = (3 * n + w) * C
                            mm = tensor.matmul(
                                cps[k % 4][:, :],
                                Wc16[:, col : col + C],
                                X2[:, s % SLOTS, g * GN + w : g * GN + w + GN],
                                start=False,
                                stop=(w == 2),
                            )
                        mm.then_inc(pe_sem, 1)
                        k += 1
            # FC + dot tail
            tensor.wait_ge(act_sem, 1)
            tensor.matmul(fps[0][:, :], Wf[:, 0:O], Hr[:, 0, :], start=True, stop=True)
            tensor.matmul(
                fps[1][:, :], Wf[:, O : 2 * O], Hr[:, 1, :], start=True, stop=True
            ).then_inc(fc_sem, 1)
            tensor.wait_ge(red_sem, 2 * NG * NSLAB + 1)
            tensor.matmul(dps[:, :], ones[:, :], P[:, :], start=True, stop=True).then_inc(
                fc_sem, 1
            )

        @block.vector
        def _(vector):
            vector.memset(ones[:, :], 1.0)
            k = 0
            for s in range(NSLAB):
                for g in range(NG):
                    p = s * NG + g  # batch pair index
                    for n in range(2):
                        vector.wait_ge(pe_sem, k + 1)
                        vector.reduce_max(
                            H[:, n, 2 * p : 2 * p + 1],
                            cps[k % 4][:, 0:VALID],
                            axis=mybir.AxisListType.X,
                        )
                        vector.reduce_max(
                            H[:, n, 2 * p + 1 : 2 * p + 2],
                            cps[k % 4][:, 200 : 200 + VALID],
                            axis=mybir.AxisListType.X,
                        ).then_inc(red_sem, 1)
                        k += 1
            vector.wait_ge(act_sem, 2)
            vector.tensor_mul(P[:, :], Ofc[:, 0, :], Ofc[:, 1, :]).then_inc(red_sem, 1)

        @block.scalar
        def _(scalar):
            # second DGE queue: fp16 leftover planes (+ disjoint boot slices)
            if BOOT:
                scalar.dma_start(out=X2[:, 0, 0:402], in_=xp2[:, 0:402]).then_inc(dma2_sem, 16)
            scalar.dma_start(out=Wc16[:, :], in_=wst16[:, :]).then_inc(dma2_sem, 16)
            for s in range(NSLAB):
                if s >= SLOTS:
                    scalar.wait_ge(pe_sem, 2 * NG * (s - SLOTS + 1))
                lo = 402 if (s == 0 and BOOT) else 0
                scalar.dma_start(
                    out=X2[:, s % SLOTS, lo:SLABW],
                    in_=xp2[:, s * SLAB + lo : s * SLAB + SLABW],
                ).then_inc(dma2_sem, 16)
            scalar.wait_ge(red_sem, 2 * NG * NSLAB)
            scalar.wait_ge(dma_sem, 160)
            scalar.activation(
                Hr[:, 0, :], H[:, 0, :], mybir.ActivationFunctionType.Relu,
                bias=Bc[:, 0:1], scale=1.0 / WSCALE,
            )
            scalar.activation(
                Hr[:, 1, :], H[:, 1, :], mybir.ActivationFunctionType.Relu,
                bias=Bc[:, 1:2], scale=1.0 / WSCALE,
            ).then_inc(act_sem, 1)
            scalar.wait_ge(fc_sem, 1)
            scalar.activation(
                Ofc[:, 0, :], fps[0][:, :], mybir.ActivationFunctionType.Relu,
                bias=Bf[:, 0:1],
            )
            scalar.activation(
                Ofc[:, 1, :], fps[1][:, :], mybir.ActivationFunctionType.Relu,
                bias=Bf[:, 1:2],
            ).then_inc(act_sem, 1)
            scalar.wait_ge(fc_sem, 2)
            scalar.copy(osb[:, :], dps[:, :])
            scalar.dma_start(out=out[:, :], in_=osb[:, :]).then_inc(dma2_sem, 16)

    return nc


def _prep_weights(w_conv1, w_conv2, w_fc1, w_fc2, b_conv1, b_conv2, b_fc1, b_fc2):
    wst8 = np.zeros((128, 6, 2, C), dtype=np.float32)
    wst16 = np.zeros((88, 6, C), dtype=np.float32)
    for n, wc in enumerate((w_conv1, w_conv2)):
        wcs = wc[:, 0] * WSCALE  # [C, W, E]
        for w in range(W):
            blk = 3 * n + w
            wst8[:, blk, 0, :] = wcs[:, w, 0:128].T
            wst8[:, blk, 1, :] = wcs[:, w, 128:256].T
            if n == 0:
                wst16[0:44, blk, :] = wcs[:, w, 256:300].T
            else:
                wst16[44:88, blk, :] = wcs[:, w, 256:300].T
    wf = np.concatenate([w_fc1.T, w_fc2.T], axis=1).astype(np.float32)  # [C, 2O]
    bc = np.stack([b_conv1, b_conv2], axis=1).astype(np.float32)  # [C, 2]
    bf = np.stack([b_fc1, b_fc2], axis=1).astype(np.float32)  # [O, 2]
    return (
        wst8.astype(NP_F8),
        wst16.reshape(88, 6 * C).astype(np.float16),
        wf, bc, bf,
    )


def kernel(x1, x2, w_conv1, b_conv1, w_fc1, b_fc1, w_conv2, b_conv2, w_fc2, b_fc2):
    global LAST_RESULT
    x1 = np.asarray(x1, dtype=np.float32)
    x2 = np.asarray(x2, dtype=np.float32)
    wst8, wst16, wf, bc, bf = _prep_weights(
        np.asarray(w_conv1, np.float32), np.asarray(w_conv2, np.float32),
        np.asarray(w_fc1, np.float32), np.asarray(w_fc2, np.float32),
        np.asarray(b_conv1, np.float32), np.asarray(b_conv2, np.float32),
        np.asarray(b_fc1, np.float32), np.asarray(b_fc2, np.float32),
    )

    if "nc" not in _NC_CACHE:
        _NC_CACHE["nc"] = _build_nc()
    nc = _NC_CACHE["nc"]

    in_maps = []
    for c in range(NCORES):
        xs1 = x1[c * BS : (c + 1) * BS, 0].reshape(POS, E).T  # [300, POS] f32
        xs2 = x2[c * BS : (c + 1) * BS, 0].reshape(POS, E).T
        xq8 = np.zeros((128, 2, 2, PADPOS), dtype=NP_F8)
        for n, xs in enumerate((xs1, xs2)):
            xq8[:, n, 0, :POS] = xs[0:128].astype(NP_F8)
            xq8[:, n, 1, :POS] = xs[128:256].astype(NP_F8)
        xp2 = np.zeros((88, PADPOS), dtype=np.float16)
        xp2[0:44, :POS] = xs1[256:300]
        xp2[44:88, :POS] = xs2[256:300]
        in_maps.append(
            {"xq8": xq8, "xp2": xp2, "wst8": wst8, "wst16": wst16,
             "wf": wf, "bc": bc, "bf": bf}
        )

    res = run_bass_kernel_spmd(nc, in_maps, list(range(NCORES)), trace=TRACE)
    LAST_RESULT = res
    return np.concatenate(
        [res.results[c]["out"].reshape(BS, 1) for c in range(NCORES)], axis=0
    )
